# revision 3
# baseline (speedup 1.0000x reference)
"""MultiLoraLinear Trainium2 kernel.

Problem: x [8, 2048, 4096] f32, adapter_ids [8] int, weight [16, 64, 4096] f32
         out[b] = x[b] @ weight[adapter_ids[b]].T         -> [8, 2048, 64] f32

Sharding: data-parallel over batch. B == n_cores == 8, so each NeuronCore owns
one batch element. The adapter gather (MoE routing) happens on host: each core
receives only the single [64, 4096] adapter it needs, pre-transposed/tiled.

Per-core compute: out [2048, 64] = x_b [2048, 4096] @ wT [4096, 64].
This is DMA-bound, so both operands stream as SINGLE fp16 planes:
x is 16 MB/core (~47 us at the 358 GB/s HBM-per-NC limit), w is 0.5 MB.
fp16's 10-bit mantissa gives ~2-3e-4 relative error on the output
(rounding ~2^-11 per element, accumulated over IN=4096 random-sign terms)
— two orders of magnitude inside the 2e-2 gate. PE time is ~17-27 us
(fp16 streams at bf16 rate, 1 moving row/cycle), well under the DMA floor.

The PE contracts along the partition dim, so x is host-pre-tiled IN-major:
xt[kc, p, c, s] (kc = K-chunk, p = IN%128 partition, c = chunk idx, s =
sequence). Each K-chunk is one fully contiguous 512 KB DMA with 4*CH KB
per partition line.

Matmuls: stationary = w chunk [128, 64], moving = x chunk [128, 512],
accumulated over the 32 K-chunks into one PSUM bank per s4 quarter.
Output copies PSUM -> SBUF on the scalar engine and DMAs out on the
vector engine's ring so the next rep's x stream (sync-engine HWDGE ring)
is never queued behind output drain.
"""

import numpy as np

import concourse.bass as bass
import concourse.tile as tile
from concourse import mybir
from concourse import bass_utils

B, S, IN, OUT, L = 8, 2048, 4096, 64, 16
N_CORES = 8
P = 128
KO = IN // P     # 32 contraction chunks of 128
CH = 1           # K-chunks per DMA (512 KB per transfer)
NCH = KO // CH
S4 = S // 512    # moving-dim chunks of 512 (PSUM bank limit)

F32 = mybir.dt.float32
F16 = mybir.dt.float16


def _split_sync_waits(nc):
    """walrus in this image supports very few sem-wait slots per instruction
    (fp32 Matmult rejects even 2). Move excess waits onto InstEventSemaphore
    carriers inserted immediately before the instruction on the same engine —
    same program point, so ordering semantics are unchanged."""
    counter = [0]

    def _carrier(engine, wait):
        counter[0] += 1
        e = mybir.InstEventSemaphore(name=f"wsplit-{counter[0]}", ins=[], outs=[])
        e.engine = engine
        e.sync_info = mybir.SyncInfo(on_wait=[wait], on_update=[])
        return e

    for f in nc.m.functions:
        for bb in f.blocks:
            new_insts = []
            for inst in bb.instructions:
                si = inst.sync_info
                waits = list(si.on_wait) if si and si.on_wait else []
                cap = 0 if isinstance(inst, mybir.InstMatmult) else 1
                if len(waits) > cap:
                    keep = waits[:cap]
                    for w in waits[cap:]:
                        c = _carrier(inst.engine, w)
                        nc.register_instruction(c, overwrite=True)
                        new_insts.append(c)
                    inst.sync_info = mybir.SyncInfo(
                        on_wait=keep, on_update=list(si.on_update or [])
                    )
                new_insts.append(inst)
            bb.instructions[:] = new_insts


def build_nc(n_rep: int = 1, x_bufs: int = 4):
    """Build the per-core Bass program. n_rep > 1 wraps the computation in a
    hardware For_i loop (same I/O, output overwritten) so harnesses can
    measure steady-state HW time by wall-clock slope; grading uses n_rep=1."""
    nc = bass.Bass("TRN2", target_bir_lowering=False, debug=False)
    x_ap = nc.dram_tensor("xt", [NCH, P, CH, S], F16, kind="ExternalInput").ap()
    w_ap = nc.dram_tensor("wt", [P, KO, OUT], F16, kind="ExternalInput").ap()
    o_ap = nc.dram_tensor("out", [OUT, S], F32, kind="ExternalOutput").ap()

    with tile.TileContext(nc) as tc:
        with (
            tc.tile_pool(name="wpool", bufs=1) as wpool,
            tc.tile_pool(name="xpool", bufs=x_bufs) as xpool,
            tc.tile_pool(name="opool", bufs=2) as opool,
            tc.tile_pool(name="pspool", bufs=1, space="PSUM") as pspool,
        ):
            w_sb = wpool.tile([P, KO, OUT], F16)
            # SWDGE ring for the 0.5 MB weight preload so the x stream starts
            # immediately on the HWDGE ring.
            nc.gpsimd.dma_start(w_sb[:], w_ap[:])

            def body():
                pss = [
                    pspool.tile([OUT, 512], F32, tag=f"ps{s4}", name=f"ps{s4}")
                    for s4 in range(S4)
                ]
                for kc in range(NCH):
                    xt = xpool.tile([P, CH, S], F16, tag="xt")
                    nc.sync.dma_start(xt[:], x_ap[kc])
                    for c in range(CH):
                        ko = kc * CH + c
                        w_k = w_sb[:, ko, :]       # [128, OUT]
                        for s4 in range(S4):
                            xs = xt[:, c, s4 * 512:(s4 + 1) * 512]
                            nc.tensor.matmul(
                                pss[s4][:, :], w_k, xs,
                                start=(ko == 0), stop=(ko == KO - 1),
                                skip_group_check=True,
                            )
                for s4 in range(S4):
                    ot = opool.tile([OUT, 512], F32, tag="ot")
                    nc.scalar.copy(ot[:], pss[s4][:, :])
                    nc.vector.dma_start(o_ap[:, s4 * 512:(s4 + 1) * 512], ot[:])

            if n_rep == 1:
                body()
            else:
                with tc.For_i(0, n_rep, 1):
                    body()
    _split_sync_waits(nc)
    return nc


def make_in_maps(x: np.ndarray, adapter_ids: np.ndarray, weight: np.ndarray):
    """Host-side sharding: per-core adapter gather + fp16 conversion + tiling.

    xt[kc, p, c, s] = fp16(x[b, s, (kc*CH+c)*128 + p])
    wt[p, ko, o]    = fp16(weight[id_b, o, ko*128 + p])
    """
    x = np.asarray(x, dtype=np.float32)
    ids = np.asarray(adapter_ids).astype(np.int64)
    w = np.asarray(weight, dtype=np.float32)

    # vectorized across the batch: one transpose + one fp16 cast for all cores
    xa = np.ascontiguousarray(x.transpose(0, 2, 1)).reshape(B, KO, P, S)
    xh = xa.astype(np.float16)                             # [B, KO, P, S]
    xh = np.ascontiguousarray(
        xh.reshape(B, NCH, CH, P, S).transpose(0, 1, 3, 2, 4)
    )                                                      # [B, NCH, P, CH, S]

    wsel = w[ids]                                          # [B, OUT, IN]
    wt = np.ascontiguousarray(wsel.transpose(0, 2, 1)).reshape(B, KO, P, OUT)
    wt = np.ascontiguousarray(wt.transpose(0, 2, 1, 3)).astype(np.float16)
    # [B, P, KO, OUT]

    return [{"xt": xh[b], "wt": wt[b]} for b in range(B)]


_NC_CACHE = {}


def kernel(x, adapter_ids, weight):
    x = np.asarray(x)
    assert x.shape == (B, S, IN), x.shape
    if "nc" not in _NC_CACHE:
        _NC_CACHE["nc"] = build_nc()
    nc = _NC_CACHE["nc"]
    in_maps = make_in_maps(x, adapter_ids, weight)
    res = bass_utils.run_bass_kernel_spmd(
        nc, in_maps, core_ids=list(range(N_CORES)), trace=False
    )
    out = np.stack(
        [res.results[b]["out"].T for b in range(B)], axis=0
    )
    return np.ascontiguousarray(out, dtype=np.float32)


# revision 4
# speedup vs baseline: 1.6362x; 1.6362x over previous
"""MultiLoraLinear Trainium2 kernel.

Problem: x [8, 2048, 4096] f32, adapter_ids [8] int, weight [16, 64, 4096] f32
         out[b] = x[b] @ weight[adapter_ids[b]].T         -> [8, 2048, 64] f32

Sharding: data-parallel over batch. B == n_cores == 8, so each NeuronCore owns
one batch element. The adapter gather (MoE routing) happens on host: each core
receives only the single [64, 4096] adapter it needs, pre-transposed/tiled.

Per-core compute: out [2048, 64] = x_b [2048, 4096] @ wT [4096, 64].
This is DMA-bound, so both operands stream as SINGLE fp16 planes:
x is 16 MB/core (~47 us at the 358 GB/s HBM-per-NC limit), w is 0.5 MB.
fp16's 10-bit mantissa gives ~2-3e-4 relative error on the output
(rounding ~2^-11 per element, accumulated over IN=4096 random-sign terms)
— two orders of magnitude inside the 2e-2 gate. PE time is ~17-27 us
(fp16 streams at bf16 rate, 1 moving row/cycle), well under the DMA floor.

The PE contracts along the partition dim, so x is host-pre-tiled IN-major:
xt[kc, p, c, s] (kc = K-chunk, p = IN%128 partition, c = chunk idx, s =
sequence). Each K-chunk is one fully contiguous 512 KB DMA with 4*CH KB
per partition line.

Matmuls: stationary = w chunk [128, 64], moving = x chunk [128, 512],
accumulated over the 32 K-chunks into one PSUM bank per s4 quarter.
Output copies PSUM -> SBUF on the scalar engine and DMAs out on the
vector engine's ring so the next rep's x stream (sync-engine HWDGE ring)
is never queued behind output drain.
"""

import numpy as np

import concourse.bass as bass
import concourse.tile as tile
from concourse import mybir
from concourse import bass_utils

B, S, IN, OUT, L = 8, 2048, 4096, 64, 16
N_CORES = 8
P = 128
KO = IN // P     # 32 contraction chunks of 128
CH = 1           # K-chunks per DMA (512 KB per transfer)
NCH = KO // CH
S4 = S // 512    # moving-dim chunks of 512 (PSUM bank limit)

F32 = mybir.dt.float32
F16 = mybir.dt.float16


def _split_sync_waits(nc):
    """walrus in this image supports very few sem-wait slots per instruction
    (fp32 Matmult rejects even 2). Move excess waits onto InstEventSemaphore
    carriers inserted immediately before the instruction on the same engine —
    same program point, so ordering semantics are unchanged."""
    counter = [0]

    def _carrier(engine, wait):
        counter[0] += 1
        e = mybir.InstEventSemaphore(name=f"wsplit-{counter[0]}", ins=[], outs=[])
        e.engine = engine
        e.sync_info = mybir.SyncInfo(on_wait=[wait], on_update=[])
        return e

    for f in nc.m.functions:
        for bb in f.blocks:
            new_insts = []
            for inst in bb.instructions:
                si = inst.sync_info
                waits = list(si.on_wait) if si and si.on_wait else []
                cap = 0 if isinstance(inst, mybir.InstMatmult) else 1
                if len(waits) > cap:
                    keep = waits[:cap]
                    for w in waits[cap:]:
                        c = _carrier(inst.engine, w)
                        nc.register_instruction(c, overwrite=True)
                        new_insts.append(c)
                    inst.sync_info = mybir.SyncInfo(
                        on_wait=keep, on_update=list(si.on_update or [])
                    )
                new_insts.append(inst)
            bb.instructions[:] = new_insts


def build_nc(n_rep: int = 1, x_bufs: int = 4):
    """Build the per-core Bass program. n_rep > 1 wraps the computation in a
    hardware For_i loop (same I/O, output overwritten) so harnesses can
    measure steady-state HW time by wall-clock slope; grading uses n_rep=1."""
    nc = bass.Bass("TRN2", target_bir_lowering=False, debug=False)
    x_ap = nc.dram_tensor("xt", [NCH, P, CH, S], F16, kind="ExternalInput").ap()
    w_ap = nc.dram_tensor("wt", [P, KO, OUT], F16, kind="ExternalInput").ap()
    o_ap = nc.dram_tensor("out", [OUT, S], F32, kind="ExternalOutput").ap()

    with tile.TileContext(nc) as tc:
        with (
            tc.tile_pool(name="wpool", bufs=1) as wpool,
            tc.tile_pool(name="xpool", bufs=x_bufs) as xpool,
            tc.tile_pool(name="opool", bufs=2) as opool,
            tc.tile_pool(name="pspool", bufs=1, space="PSUM") as pspool,
        ):
            w_sb = wpool.tile([P, KO, OUT], F16)
            # SWDGE ring for the 0.5 MB weight preload so the x stream starts
            # immediately on the HWDGE ring.
            nc.gpsimd.dma_start(w_sb[:], w_ap[:])

            def body():
                pss = [
                    pspool.tile([OUT, 512], F32, tag=f"ps{s4}", name=f"ps{s4}")
                    for s4 in range(S4)
                ]
                for kc in range(NCH):
                    xt = xpool.tile([P, CH, S], F16, tag="xt")
                    nc.sync.dma_start(xt[:], x_ap[kc])
                    for c in range(CH):
                        ko = kc * CH + c
                        w_k = w_sb[:, ko, :]       # [128, OUT]
                        for s4 in range(S4):
                            xs = xt[:, c, s4 * 512:(s4 + 1) * 512]
                            nc.tensor.matmul(
                                pss[s4][:, :], w_k, xs,
                                start=(ko == 0), stop=(ko == KO - 1),
                                skip_group_check=True,
                            )
                for s4 in range(S4):
                    ot = opool.tile([OUT, 512], F32, tag="ot")
                    nc.scalar.copy(ot[:], pss[s4][:, :])
                    nc.scalar.dma_start(o_ap[:, s4 * 512:(s4 + 1) * 512], ot[:])

            if n_rep == 1:
                body()
            else:
                with tc.For_i(0, n_rep, 1):
                    body()
    _split_sync_waits(nc)
    return nc


def make_in_maps(x: np.ndarray, adapter_ids: np.ndarray, weight: np.ndarray):
    """Host-side sharding: per-core adapter gather + fp16 conversion + tiling.

    xt[kc, p, c, s] = fp16(x[b, s, (kc*CH+c)*128 + p])
    wt[p, ko, o]    = fp16(weight[id_b, o, ko*128 + p])
    """
    x = np.asarray(x, dtype=np.float32)
    ids = np.asarray(adapter_ids).astype(np.int64)
    w = np.asarray(weight, dtype=np.float32)

    # vectorized across the batch: one transpose + one fp16 cast for all cores
    xa = np.ascontiguousarray(x.transpose(0, 2, 1)).reshape(B, KO, P, S)
    xh = xa.astype(np.float16)                             # [B, KO, P, S]
    xh = np.ascontiguousarray(
        xh.reshape(B, NCH, CH, P, S).transpose(0, 1, 3, 2, 4)
    )                                                      # [B, NCH, P, CH, S]

    wsel = w[ids]                                          # [B, OUT, IN]
    wt = np.ascontiguousarray(wsel.transpose(0, 2, 1)).reshape(B, KO, P, OUT)
    wt = np.ascontiguousarray(wt.transpose(0, 2, 1, 3)).astype(np.float16)
    # [B, P, KO, OUT]

    return [{"xt": xh[b], "wt": wt[b]} for b in range(B)]


_NC_CACHE = {}


def kernel(x, adapter_ids, weight):
    x = np.asarray(x)
    assert x.shape == (B, S, IN), x.shape
    if "nc" not in _NC_CACHE:
        _NC_CACHE["nc"] = build_nc()
    nc = _NC_CACHE["nc"]
    in_maps = make_in_maps(x, adapter_ids, weight)
    res = bass_utils.run_bass_kernel_spmd(
        nc, in_maps, core_ids=list(range(N_CORES)), trace=False
    )
    out = np.stack(
        [res.results[b]["out"].T for b in range(B)], axis=0
    )
    return np.ascontiguousarray(out, dtype=np.float32)


# revision 5
# speedup vs baseline: 1.7770x; 1.0860x over previous
"""MultiLoraLinear Trainium2 kernel.

Problem: x [8, 2048, 4096] f32, adapter_ids [8] int, weight [16, 64, 4096] f32
         out[b] = x[b] @ weight[adapter_ids[b]].T         -> [8, 2048, 64] f32

Sharding: data-parallel over batch. B == n_cores == 8, so each NeuronCore owns
one batch element. The adapter gather (MoE routing) happens on host: each core
receives only the single [64, 4096] adapter it needs, pre-transposed/tiled.

Per-core compute: out [2048, 64] = x_b [2048, 4096] @ wT [4096, 64].
This is DMA-bound, so both operands stream as SINGLE fp16 planes:
x is 16 MB/core (~47 us at the 358 GB/s HBM-per-NC limit), w is 0.5 MB.
fp16's 10-bit mantissa gives ~2-3e-4 relative error on the output
(rounding ~2^-11 per element, accumulated over IN=4096 random-sign terms)
— two orders of magnitude inside the 2e-2 gate. PE time is ~17-27 us
(fp16 streams at bf16 rate, 1 moving row/cycle), well under the DMA floor.

The PE contracts along the partition dim, so x is host-pre-tiled IN-major:
xt[kc, p, c, s] (kc = K-chunk, p = IN%128 partition, c = chunk idx, s =
sequence). Each K-chunk is one fully contiguous 512 KB DMA with 4*CH KB
per partition line.

Matmuls: stationary = w chunk [128, 64], moving = x chunk [128, 512],
accumulated over the 32 K-chunks into one PSUM bank per s4 quarter.
Output copies PSUM -> SBUF on the scalar engine and DMAs out on the
vector engine's ring so the next rep's x stream (sync-engine HWDGE ring)
is never queued behind output drain.
"""

import numpy as np

import concourse.bass as bass
import concourse.tile as tile
from concourse import mybir
from concourse import bass_utils

B, S, IN, OUT, L = 8, 2048, 4096, 64, 16
N_CORES = 8
P = 128
KO = IN // P     # 32 contraction chunks of 128
CH = 4           # K-chunks per DMA (2 MB per transfer, 16 KB per partition line)
NCH = KO // CH
S4 = S // 512    # moving-dim chunks of 512 (PSUM bank limit)

F32 = mybir.dt.float32
F16 = mybir.dt.float16


def _split_sync_waits(nc):
    """walrus in this image supports very few sem-wait slots per instruction
    (fp32 Matmult rejects even 2). Move excess waits onto InstEventSemaphore
    carriers inserted immediately before the instruction on the same engine —
    same program point, so ordering semantics are unchanged."""
    counter = [0]

    def _carrier(engine, wait):
        counter[0] += 1
        e = mybir.InstEventSemaphore(name=f"wsplit-{counter[0]}", ins=[], outs=[])
        e.engine = engine
        e.sync_info = mybir.SyncInfo(on_wait=[wait], on_update=[])
        return e

    for f in nc.m.functions:
        for bb in f.blocks:
            new_insts = []
            for inst in bb.instructions:
                si = inst.sync_info
                waits = list(si.on_wait) if si and si.on_wait else []
                cap = 0 if isinstance(inst, mybir.InstMatmult) else 1
                if len(waits) > cap:
                    keep = waits[:cap]
                    for w in waits[cap:]:
                        c = _carrier(inst.engine, w)
                        nc.register_instruction(c, overwrite=True)
                        new_insts.append(c)
                    inst.sync_info = mybir.SyncInfo(
                        on_wait=keep, on_update=list(si.on_update or [])
                    )
                new_insts.append(inst)
            bb.instructions[:] = new_insts


def build_nc(n_rep: int = 1, x_bufs: int = 6):
    """Build the per-core Bass program. n_rep > 1 wraps the computation in a
    hardware For_i loop (same I/O, output overwritten) so harnesses can
    measure steady-state HW time by wall-clock slope; grading uses n_rep=1."""
    nc = bass.Bass("TRN2", target_bir_lowering=False, debug=False)
    x_ap = nc.dram_tensor("xt", [NCH, P, CH, S], F16, kind="ExternalInput").ap()
    w_ap = nc.dram_tensor("wt", [P, KO, OUT], F16, kind="ExternalInput").ap()
    o_ap = nc.dram_tensor("out", [OUT, S], F32, kind="ExternalOutput").ap()

    with tile.TileContext(nc) as tc:
        with (
            tc.tile_pool(name="wpool", bufs=1) as wpool,
            tc.tile_pool(name="xpool", bufs=x_bufs) as xpool,
            tc.tile_pool(name="opool", bufs=2) as opool,
            tc.tile_pool(name="pspool", bufs=1, space="PSUM") as pspool,
        ):
            w_sb = wpool.tile([P, KO, OUT], F16)
            # SWDGE ring for the 0.5 MB weight preload so the x stream starts
            # immediately on the HWDGE ring.
            nc.gpsimd.dma_start(w_sb[:], w_ap[:])

            def body():
                pss = [
                    pspool.tile([OUT, 512], F32, tag=f"ps{s4}", name=f"ps{s4}")
                    for s4 in range(S4)
                ]
                for kc in range(NCH):
                    xt = xpool.tile([P, CH, S], F16, tag="xt")
                    nc.sync.dma_start(xt[:], x_ap[kc])
                    for c in range(CH):
                        ko = kc * CH + c
                        w_k = w_sb[:, ko, :]       # [128, OUT]
                        for s4 in range(S4):
                            xs = xt[:, c, s4 * 512:(s4 + 1) * 512]
                            nc.tensor.matmul(
                                pss[s4][:, :], w_k, xs,
                                start=(ko == 0), stop=(ko == KO - 1),
                                skip_group_check=True,
                            )
                for s4 in range(S4):
                    ot = opool.tile([OUT, 512], F32, tag="ot")
                    nc.scalar.copy(ot[:], pss[s4][:, :])
                    nc.scalar.dma_start(o_ap[:, s4 * 512:(s4 + 1) * 512], ot[:])

            if n_rep == 1:
                body()
            else:
                with tc.For_i(0, n_rep, 1):
                    body()
    _split_sync_waits(nc)
    return nc


def make_in_maps(x: np.ndarray, adapter_ids: np.ndarray, weight: np.ndarray):
    """Host-side sharding: per-core adapter gather + fp16 conversion + tiling.

    xt[kc, p, c, s] = fp16(x[b, s, (kc*CH+c)*128 + p])
    wt[p, ko, o]    = fp16(weight[id_b, o, ko*128 + p])
    """
    x = np.asarray(x, dtype=np.float32)
    ids = np.asarray(adapter_ids).astype(np.int64)
    w = np.asarray(weight, dtype=np.float32)

    # vectorized across the batch: one transpose + one fp16 cast for all cores
    xa = np.ascontiguousarray(x.transpose(0, 2, 1)).reshape(B, KO, P, S)
    xh = xa.astype(np.float16)                             # [B, KO, P, S]
    xh = np.ascontiguousarray(
        xh.reshape(B, NCH, CH, P, S).transpose(0, 1, 3, 2, 4)
    )                                                      # [B, NCH, P, CH, S]

    wsel = w[ids]                                          # [B, OUT, IN]
    wt = np.ascontiguousarray(wsel.transpose(0, 2, 1)).reshape(B, KO, P, OUT)
    wt = np.ascontiguousarray(wt.transpose(0, 2, 1, 3)).astype(np.float16)
    # [B, P, KO, OUT]

    return [{"xt": xh[b], "wt": wt[b]} for b in range(B)]


_NC_CACHE = {}


def kernel(x, adapter_ids, weight):
    x = np.asarray(x)
    assert x.shape == (B, S, IN), x.shape
    if "nc" not in _NC_CACHE:
        _NC_CACHE["nc"] = build_nc()
    nc = _NC_CACHE["nc"]
    in_maps = make_in_maps(x, adapter_ids, weight)
    res = bass_utils.run_bass_kernel_spmd(
        nc, in_maps, core_ids=list(range(N_CORES)), trace=False
    )
    out = np.stack(
        [res.results[b]["out"].T for b in range(B)], axis=0
    )
    return np.ascontiguousarray(out, dtype=np.float32)
